# revision 1
# baseline (speedup 1.0000x reference)
"""Trainium2 Bass kernel for sparse (strided) multi-head attention.

Reference computation (B=2, S=2048, H=1024, NH=16, D=64):
    q = (x @ q_w) * sigmoid(phi); k = x @ k_w; v = x @ v_w   (per-head [S, D])
    scores = q k^T / sqrt(D), masked to allowed[i, j] = (j % 4 == 0) | (|i-j| <= 8)
    out = softmax(scores) @ v;  return concat_heads(out) @ o_w + o_b

Sharding: 8 cores = 2 batches x 4 head-groups (4 heads each). Each core gets
x^T for its batch, column-sliced q/k/v weights, row-sliced o_w, and returns a
partial transposed output F^T = (attn_out_heads @ o_w_slice)^T which the host
sums over head-groups, transposes, and biases.

Device algorithm per core (all scores computed transposed: [key, query]):
  - QKV projections from x^T (contraction over H on 128-partitions).
    Q^T/K^T stored D-major ([2 c-tiles of 128ch] x S); V stored S-major with a
    ones column appended (col 64) so the attn@V matmul also emits row-sums.
  - Sparse structure: strided part (keys j%4==0 -> 512 keys, no mask), 16
    diagonal 128x128 blocks (band mask, strided keys excluded), 15x2 corner
    pieces for the +/-8 band crossing tile boundaries.
  - exp on ScalarE (no max subtraction needed: scores are O(10)), additive
    masks injected into PSUM via identity-weight matmuls.
  - attn@V accumulated in PSUM [65, 1024] per query-half; row 64 = softmax
    denominators; normalization = reciprocal + row-broadcast (DMA) + multiply.
  - F^T = o_w_slice^T-contraction against normalized head outputs.
"""

import os
import numpy as np

B, S, H = 2, 2048, 1024
NH, D = 16, 64
PHI = 1.6180339887
STRIDE, LOCAL = 4, 8
HPG = 4              # heads per group (= per core)
GC = HPG * D         # channels per core = 256
NKT = S // 128       # 16 key tiles
NSK = S // STRIDE    # 512 strided keys
MASKVAL = -30000.0

_CACHE = {}
LAST_RESULTS = None  # BassKernelResults of the most recent run (for profiling)


def _mm_dtype():
    import concourse.mybir as mybir
    name = os.environ.get("KERNEL_MM_DTYPE", "float32r")
    return {"float32": mybir.dt.float32, "float32r": mybir.dt.float32r}[name]


def host_masks():
    """Static additive masks (0 = allowed, MASKVAL = disallowed)."""
    k = np.arange(128)[:, None]
    q = np.arange(128)[None, :]
    # multiplicative 0/1 mask, applied to exp'd diagonal-block scores on DVE
    maskd = np.where((np.abs(q - k) <= LOCAL) & (k % STRIDE != 0), 1.0, 0.0)

    r = np.arange(16)[:, None]
    c = np.arange(8)[None, :]
    # multiplicative 0/1 corner masks (applied post-exp on DVE)
    # piece A: q in [b-8, b), k = b-8+r ; allowed iff k >= b, k-q <= 8, k%4 != 0
    mca = np.where((r >= 8) & (r - c <= 8) & (r % STRIDE != 0), 1.0, 0.0)
    # piece B: q in [b, b+8), k = b-8+r ; allowed iff k < b, q-k <= 8, k%4 != 0
    mcb = np.where((r < 8) & (c <= r) & (r % STRIDE != 0), 1.0, 0.0)
    maskc = np.concatenate([mca, mcb], axis=1)  # [16, 16]
    return maskd.astype(np.float32), maskc.astype(np.float32)


def build_nc(loop_n=1):
    """Build the per-core Bass program (same NEFF for all 8 cores).

    loop_n > 1 wraps the whole pipeline in a hardware loop (benchmarking:
    wall-clock deltas between loop counts cancel dispatch overhead).
    """
    import contextlib
    import concourse.bass as bass
    import concourse.mybir as mybir
    import concourse.tile as tile
    from concourse import bacc

    f32 = mybir.dt.float32
    DT = _mm_dtype()
    AF = mybir.ActivationFunctionType

    nc = bacc.Bacc("TRN2", target_bir_lowering=False, debug=False)

    d_xT = nc.dram_tensor("xT", [H, S], DT, kind="ExternalInput")
    d_qw = nc.dram_tensor("qw", [H, GC], DT, kind="ExternalInput")
    d_kw = nc.dram_tensor("kw", [H, GC], DT, kind="ExternalInput")
    d_vw = nc.dram_tensor("vw", [H, GC], DT, kind="ExternalInput")
    d_ow = nc.dram_tensor("ow", [GC, H], DT, kind="ExternalInput")
    d_maskd = nc.dram_tensor("maskd", [128, 128], DT, kind="ExternalInput")
    d_maskc = nc.dram_tensor("maskc", [16, 16], DT, kind="ExternalInput")
    d_ident = nc.dram_tensor("ident", [128, 128], DT, kind="ExternalInput")
    d_ones = nc.dram_tensor("ones", [128, 80], DT, kind="ExternalInput")
    d_fT = nc.dram_tensor("fT", [H, S], f32, kind="ExternalOutput")
    d_scr = nc.dram_tensor("nrm_scr", [HPG * 2, 1024], f32)  # rowsum bounce

    def mm(out, lhsT, rhs, start, stop, tile_position=None):
        nc.tensor.matmul(out, lhsT, rhs, start=start, stop=stop,
                         skip_group_check=True, tile_position=tile_position)

    with tile.TileContext(nc) as tc:
        with (
            tc.tile_pool(name="consts", bufs=1) as consts,
            tc.tile_pool(name="persist", bufs=1) as persist,
        ):
            sb_ident = consts.tile([128, 128], DT)
            sb_maskd = consts.tile([128, 128], DT)
            sb_maskc = consts.tile([16, 16], DT)
            sb_ow = persist.tile([128, 2, 1024], DT)

            # D-major Q^T / K^T: [128ch (2 heads), c-tile, S]
            sb_QT = persist.tile([128, 2, S], DT)
            sb_KT = persist.tile([128, 2, S], DT)
            sb_KsT = persist.tile([128, 2, NSK], DT)      # strided keys, compacted
            # S-major V with ones col: [128, s-tile, head, 66] (col 64 = 1.0)
            sb_V = persist.tile([128, NKT, HPG, 66], DT)
            sb_Vs = persist.tile([128, NSK // 128, HPG, 66], DT)
            # corner V rows: k in [128t-8, 128t+8) on partitions 0-15
            # (matmul psum dst must start at partition 0)
            sb_Vc = persist.tile([16, NKT - 1, HPG, 66], DT)
            sb_outTs = persist.tile([128, 2, S], DT)      # c-major head outputs

            loop_cm = tc.For_i(0, loop_n, 1) if loop_n > 1 else contextlib.nullcontext()
            with loop_cm:
             # ---------------- Phase 1: load + QKV projections ----------------
             with (
                tc.tile_pool(name="ph1", bufs=1) as ph1,
                tc.tile_pool(name="psA", bufs=6, space="PSUM") as psA,
            ):
                sb_xT = ph1.tile([128, 8, S], DT)
                sb_qw = ph1.tile([128, 8, GC], DT)
                sb_kw = ph1.tile([128, 8, GC], DT)
                sb_vw = ph1.tile([128, 8, GC], DT)
                # critical path first: qw + x^T tiles gate the first matmuls;
                # kw/vw next; ow/masks/ident are not needed until much later
                nc.sync.dma_start(out=sb_qw[:], in_=d_qw.rearrange("(t p) c -> p t c", p=128))
                xt_r = d_xT.rearrange("(t p) s -> t p s", p=128)
                for ht in range(8):
                    nc.sync.dma_start(out=sb_xT[:, ht, :], in_=xt_r[ht])
                nc.sync.dma_start(out=sb_kw[:], in_=d_kw.rearrange("(t p) c -> p t c", p=128))
                nc.sync.dma_start(out=sb_vw[:], in_=d_vw.rearrange("(t p) c -> p t c", p=128))
                nc.sync.dma_start(out=sb_ow[:], in_=d_ow.rearrange("(t p) f -> p t f", p=128))
                nc.sync.dma_start(out=sb_ident[:], in_=d_ident[:])
                nc.sync.dma_start(out=sb_maskd[:], in_=d_maskd[:])
                nc.sync.dma_start(out=sb_maskc[:], in_=d_maskc[:])

                cp = 0  # copy-engine parity

                def psum_copy(dst, src):
                    # all copies on DVE: ScalarE is reserved for the exps,
                    # which bound the attention phase
                    nonlocal cp
                    nc.vector.tensor_copy(dst, src)
                    cp += 1

                # Q^T / K^T: waves of 4 psums, ht-outer for DMA/PE pipelining
                for w_sb, w_out in ((sb_qw, sb_QT), (sb_kw, sb_KT)):
                    for ct in range(2):
                        ps = [psA.tile([128, 512], f32, tag="proj", name="psproj") for _ in range(4)]
                        for ht in range(8):
                            for ss in range(4):
                                mm(ps[ss][:], w_sb[:, ht, 128 * ct:128 * (ct + 1)],
                                   sb_xT[:, ht, 512 * ss:512 * (ss + 1)],
                                   start=(ht == 0), stop=(ht == 7))
                        for ss in range(4):
                            psum_copy(w_out[:, ct, 512 * ss:512 * (ss + 1)], ps[ss][:])

                # V (S-major), 4 waves of 4 s-tiles
                for wv in range(4):
                    ps = [psA.tile([128, GC], f32, tag="proj", name="psprojv") for _ in range(4)]
                    for ht in range(8):
                        for j in range(4):
                            st = 4 * wv + j
                            mm(ps[j][:], sb_xT[:, ht, 128 * st:128 * (st + 1)],
                               sb_vw[:, ht, :], start=(ht == 0), stop=(ht == 7))
                    for j in range(4):
                        st = 4 * wv + j
                        psum_copy(sb_V[:, st, :, 0:64],
                                  ps[j].rearrange("p (h d) -> p h d", h=HPG))

                # V_s = V[::4]: partition-strided SBUF->SBUF DMA gather
                # (replaces 32 PE matmuls): strided key r = 128*sst + j lives
                # at V tile 4*sst + j//32, partition 4*(j%32)
                for sst in range(4):
                    for m in range(4):
                        nc.sync.dma_start(
                            out=sb_Vs[32 * m:32 * (m + 1), sst, :, 0:64],
                            in_=sb_V[0:128:4, 4 * sst + m, :, 0:64])

                # ones columns (DMA from host array: memset can't write f32r)
                nc.sync.dma_start(
                    out=sb_V[:, :, :, 64],
                    in_=d_ones[:, 0:64].rearrange("p (t h) -> p t h", h=HPG))
                nc.sync.dma_start(
                    out=sb_Vs[:, :, :, 64],
                    in_=d_ones[:, 64:80].rearrange("p (t h) -> p t h", h=HPG))

                # compact strided K^T
                for ct in range(2):
                    ks = sb_KT[:, ct, :].rearrange("p (r f) -> p r f", f=STRIDE)[:, :, 0]
                    nc.vector.tensor_copy(sb_KsT[:, ct, :], ks)

                # corner V rows: k in [128t-8, 128t+8) -> sb_Vc (DMA repartition)
                for t in range(1, NKT):
                    nc.sync.dma_start(out=sb_Vc[0:8, t - 1, :, 0:65],
                                      in_=sb_V[120:128, t - 1, :, 0:65])
                    nc.sync.dma_start(out=sb_Vc[8:16, t - 1, :, 0:65],
                                      in_=sb_V[0:8, t, :, 0:65])

            # ---------------- Phase 2: attention per head ----------------
            with (
                tc.tile_pool(name="ats", bufs=5) as p_ats,
                tc.tile_pool(name="atd", bufs=2) as p_atd,
                tc.tile_pool(name="atc", bufs=2) as p_atc,
                tc.tile_pool(name="rows", bufs=2) as p_rows,
                tc.tile_pool(name="sums", bufs=2) as p_sums,
                tc.tile_pool(name="psS", bufs=2, space="PSUM") as psS,
                tc.tile_pool(name="psO", bufs=2, space="PSUM") as psO,
            ):
                for h in range(HPG):
                    ct, pb = h // 2, (h % 2) * 64
                    QT = sb_QT[pb:pb + 64, ct, :]
                    KT = sb_KT[pb:pb + 64, ct, :]
                    KsT = sb_KsT[pb:pb + 64, ct, :]

                    # strided scores + exp: per strided-key tile [128, S]
                    at_s = []
                    for i in range(4):
                        a = p_ats.tile([128, S], DT, tag="ats", name="at_s")
                        at_s.append(a)
                        for qh in range(2):
                            ps = psS.tile([128, 1024], f32, tag="sc", name="ps_sc")
                            for u in range(2):
                                mm(ps[:, 512 * u:512 * (u + 1)],
                                   KsT[:, 128 * i:128 * (i + 1)],
                                   QT[:, 1024 * qh + 512 * u:1024 * qh + 512 * (u + 1)],
                                   start=True, stop=True)
                            nc.scalar.activation(a[:, 1024 * qh:1024 * (qh + 1)], ps[:],
                                                 AF.Exp)

                    # diagonal blocks: raw scores -> exp -> multiplicative 0/1
                    # band mask on DVE (saves 16 PE mask-add matmuls per head;
                    # scores are O(4) so unmasked exp cannot overflow)
                    at_d = p_atd.tile([128, S], DT, tag="atd")
                    for qq in range(2):
                        ps = psS.tile([128, 1024], f32, tag="sc", name="ps_sc")
                        for j in range(8):
                            kt = 8 * qq + j
                            sl = ps[:, 128 * j:128 * (j + 1)]
                            mm(sl, KT[:, 128 * kt:128 * (kt + 1)],
                               QT[:, 128 * kt:128 * (kt + 1)],
                               start=(j % 4 == 0), stop=(j % 4 == 3))
                        nc.scalar.activation(at_d[:, 1024 * qq:1024 * (qq + 1)], ps[:],
                                             AF.Exp)
                        md = sb_maskd[:]
                        bcast = bass.AP(tensor=md.tensor, offset=md.offset,
                                        ap=[list(md.ap[0]), [0, 8], list(md.ap[1])])
                        sl8 = at_d[:, 1024 * qq:1024 * (qq + 1)].rearrange(
                            "p (a b) -> p a b", b=128)
                        nc.vector.tensor_mul(sl8, sl8, bcast)

                    # corner pieces: [16, 16] per boundary, 3 boundaries packed
                    # per partition-width at bases {0, 32, 64}. PSUM start/stop
                    # grouping is tracked per partition-range.
                    at_c = p_atc.tile([16, NKT - 1, 2, 8], DT, tag="atc", name="at_c")
                    # full-bank pitch (512 f32) keeps per-bank accumulation
                    # bookkeeping aligned; cols 16*(t-1)+8p, 240 of 512 used
                    psc = psS.tile([16, 512], f32, tag="sc", name="psc")
                    for t in range(1, NKT):
                        for p in range(2):
                            sl = psc[:, 16 * (t - 1) + 8 * p:16 * (t - 1) + 8 * (p + 1)]
                            mm(sl, KT[:, 128 * t - 8:128 * t + 8],
                               QT[:, 128 * t - 8 + 8 * p:128 * t + 8 * p],
                               start=(t == 1 and p == 0),
                               stop=(t == NKT - 1 and p == 1))
                    nc.scalar.activation(at_c[:], psc[:, 0:240], AF.Exp)
                    mc = sb_maskc[:]
                    cbc = bass.AP(tensor=mc.tensor, offset=mc.offset,
                                  ap=[list(mc.ap[0]), [0, NKT - 1], list(mc.ap[1])])
                    atc3 = at_c.rearrange("p a b c -> p a (b c)")
                    nc.vector.tensor_mul(atc3, atc3, cbc)

                    # attn @ [V | 1]: psum_out [65, 1024] per query half
                    for qh in range(2):
                        po = psO.tile([65, 1024], f32, tag="out", name="ps_out")
                        for r in range(2):
                            qlo = 1024 * qh + 512 * r
                            ops = []  # (out_slice, lhsT, rhs)
                            for i in range(4):
                                ops.append((po[:, 512 * r:512 * (r + 1)],
                                            sb_Vs[:, i, h, 0:65],
                                            at_s[i][:, qlo:qlo + 512]))
                            for j in range(4):
                                kt = 8 * qh + 4 * r + j
                                ops.append((po[:, 128 * (4 * r + j):128 * (4 * r + j) + 128],
                                            sb_V[:, kt, h, 0:65],
                                            at_d[:, 128 * kt:128 * kt + 128]))
                            def corner(t, p):
                                lo = 128 * t - 8 * (1 - p) - 1024 * qh
                                ops.append((po[:, lo:lo + 8],
                                            sb_Vc[:, t - 1, h, 0:65],
                                            at_c[:, t - 1, p, :]))

                            for t in range(8 * qh + 4 * r + 1, 8 * qh + 4 * r + 5):
                                if 1 <= t <= NKT - 1:  # piece A: q in [128t-8, 128t)
                                    corner(t, 0)
                            for t in range(max(1, 8 * qh + 4 * r), 8 * qh + 4 * r + 4):
                                if 1 <= t <= NKT - 1:  # piece B: q in [128t, 128t+8)
                                    corner(t, 1)
                            for oi, (o, l, rr) in enumerate(ops):
                                mm(o, l, rr, start=(oi == 0), stop=(oi == len(ops) - 1))

                        # normalize: row 64 = sums -> reciprocal -> scale rows
                        rows = p_rows.tile([65, 1024], f32, tag="rows", name="rows")
                        nc.vector.tensor_copy(rows[64:65, :], po[64:65, :])
                        sums = p_sums.tile([64, 1024], f32, tag="sums", name="sums")
                        scr = d_scr[2 * h + qh, :]
                        nc.sync.dma_start(out=scr, in_=rows[64:65, :])
                        nc.sync.dma_start(out=sums[:], in_=scr.partition_broadcast(64))
                        rec = p_sums.tile([64, 1024], f32, tag="rec", name="rec")
                        nc.vector.reciprocal(rec[:], sums[:])
                        ost = p_sums.tile([64, 1024], DT, tag="ost", name="ost")
                        nc.vector.tensor_mul(ost[:], po[0:64, :], rec[:])
                        # repartition into c-major via SBUF->SBUF DMA
                        nc.sync.dma_start(
                            out=sb_outTs[pb:pb + 64, ct, 1024 * qh:1024 * (qh + 1)],
                            in_=ost[:])

            # ---------------- Phase 3: F^T = (heads @ o_w)^T ----------------
            with (
                tc.tile_pool(name="stage", bufs=6) as p_stage,
                tc.tile_pool(name="psF", bufs=5, space="PSUM") as psF,
            ):
                cp2 = 0
                for ft in range(8):
                    for qs in range(4):
                        ps = psF.tile([128, 512], f32, tag="ft", name="ps_ft")
                        for ctt in range(2):
                            mm(ps[:], sb_ow[:, ctt, 128 * ft:128 * (ft + 1)],
                               sb_outTs[:, ctt, 512 * qs:512 * (qs + 1)],
                               start=(ctt == 0), stop=(ctt == 1))
                        st = p_stage.tile([128, 512], f32, tag="st", name="stg")
                        if cp2 % 2 == 0:
                            nc.vector.tensor_copy(st[:], ps[:])
                        else:
                            nc.scalar.copy(st[:], ps[:])
                        cp2 += 1
                        nc.sync.dma_start(
                            out=d_fT[:, 512 * qs:512 * (qs + 1)].rearrange(
                                "(t p) s -> t p s", p=128)[ft],
                            in_=st[:])

    nc.compile()
    return nc


def get_nc():
    key = os.environ.get("KERNEL_MM_DTYPE", "float32r")
    if key not in _CACHE:
        _CACHE[key] = build_nc()
    return _CACHE[key]


def host_inputs(x, q_w, k_w, v_w, o_w, o_b, unity_scale):
    """Per-core input maps."""
    sig = 1.0 / (1.0 + np.exp(-float(np.asarray(unity_scale))))
    qw_eff = (np.asarray(q_w) * (sig / np.sqrt(D))).astype(np.float32)
    xT = np.ascontiguousarray(np.asarray(x).transpose(0, 2, 1)).astype(np.float32)
    maskd, maskc = host_masks()
    ident = np.eye(128, dtype=np.float32)
    k_w = np.asarray(k_w, np.float32)
    v_w = np.asarray(v_w, np.float32)
    o_w = np.asarray(o_w, np.float32)
    in_maps = []
    for c in range(8):
        b, g = c // 4, c % 4
        cs = slice(GC * g, GC * (g + 1))
        in_maps.append({
            "xT": xT[b],
            "qw": np.ascontiguousarray(qw_eff[:, cs]),
            "kw": np.ascontiguousarray(k_w[:, cs]),
            "vw": np.ascontiguousarray(v_w[:, cs]),
            "ow": np.ascontiguousarray(o_w[cs, :]),
            "maskd": maskd, "maskc": maskc, "ident": ident,
            "ones": np.ones((128, 80), np.float32),
        })
    return in_maps


def kernel(x, q_w, k_w, v_w, o_w, o_b, unity_scale):
    global LAST_RESULTS
    from concourse.bass_utils import run_bass_kernel_spmd

    nc = get_nc()
    in_maps = host_inputs(x, q_w, k_w, v_w, o_w, o_b, unity_scale)
    res = run_bass_kernel_spmd(nc, in_maps, core_ids=list(range(8)),
                               trace=bool(os.environ.get("KERNEL_TRACE")))
    LAST_RESULTS = res
    out = np.zeros((B, S, H), np.float32)
    for b in range(B):
        acc = np.zeros((H, S), np.float32)
        for g in range(4):
            acc += res.results[4 * b + g]["fT"]
        out[b] = acc.T
    out += np.asarray(o_b, np.float32)[None, None, :]
    return out

